# revision 35
# baseline (speedup 1.0000x reference)
"""Trainium2 Bass kernel for per-(b,v)-slice masked attention.

Reference computation (per (b,v) slice, P=S=512, D=512):
    q = X_q @ Wq.T + bq          (softmax scale folded into Wq here)
    k = X_k @ Wkv.T + bkv
    v = X_v @ Wkv.T + bkv
    scores = q @ k.T, diag masked, attn = softmax(scores)
    out = (attn @ v) @ Wo.T + bo

Sharding: 128 (b,v) slices split 16-per-core across 8 cores; weights
replicated. The host pre-transposes q/k activations to d-major layout so
every on-chip matmul contracts over the partition dimension.

Fast path (bq == bkv == 0, which setup_inputs guarantees): fold the
weight products on the host —
    M  = (scale*Wq).T @ Wkv     so  scores.T = Xk @ M.T @ Xq.T
    N0 = Wkv.T @ Wo.T           so  out = (attn @ Xv) @ N0 + (Wo@bkv + bo)
eliminating the k and v projections (4 big matmul groups per slice
instead of 6). Per slice:
    u[d,p]    = (M tiles).T @ XqT
    sT[s,p]   = (XkT tiles).T @ u       (scores transposed)
    eT[s,p]   = exp(sT) * (1 - I)      (diag mask, multiplicative)
    sums[p]   = (eT tiles).T @ ones    (softmax denominator, [128,4] psum)
    axT[d,p]  = (Xv tiles).T @ eT      (attn @ Xv, transposed, unnormalized)
    out[p,o]  = ((axT tiles).T @ N0) * recip(sums)[p] + bo2

General path (nonzero bq/bkv): explicit q/k/v projections as above.

Matmul operands are bf16 (PE streams 1 cycle/row with fast weight load);
all PSUM accumulation and softmax normalization stay fp32.
"""

import numpy as np
import ml_dtypes

import concourse.bacc as bacc
import concourse.mybir as mybir
from concourse.tile import TileContext
from concourse.bass_utils import run_bass_kernel_spmd

B, V, P, D = 4, 32, 512, 512
N_CORES = 8
SLICES = B * V  # 128
SPC = SLICES // N_CORES  # 16 slices per core
KT = D // 128  # 4 contraction tiles
PT = P // 128  # 4 token tiles

BF16 = mybir.dt.bfloat16
F32 = mybir.dt.float32
AF = mybir.ActivationFunctionType
ALU = mybir.AluOpType


def _new_nc():
    return bacc.Bacc("TRN2", target_bir_lowering=False, debug=False,
                     num_devices=N_CORES)


def _load_w(nc, cpool, dram, split=False):
    t = cpool.tile([128, KT, D], BF16, tag=dram.name)
    src = dram.ap().rearrange("(kk p) f -> p kk f", p=128)
    if split:
        # per-kk pieces release matmul dependencies as each chunk lands
        for kk in range(KT):
            nc.sync.dma_start(out=t[:, kk, :], in_=src[:, kk, :])
    else:
        nc.sync.dma_start(out=t[:], in_=src)
    return t


def _load_x(nc, xpool, dram, s, tag, split=False, engine=None):
    t = xpool.tile([128, KT, P], BF16, tag=tag)
    src = dram.ap()[s].rearrange("(kk p) f -> p kk f", p=128)
    if split:
        for kk in range(KT):
            nc.sync.dma_start(out=t[:, kk, :], in_=src[:, kk, :])
    else:
        (engine or nc.sync).dma_start(out=t[:], in_=src)
    return t


def build_program_fast():
    """Zero-bias fast path: 4 matmul groups per slice."""
    nc = _new_nc()

    xq_d = nc.dram_tensor("xqT", [SPC, D, P], BF16, kind="ExternalInput")
    xk_d = nc.dram_tensor("xkT", [SPC, D, P], BF16, kind="ExternalInput")
    xv_d = nc.dram_tensor("xvN", [SPC, P, D], BF16, kind="ExternalInput")
    m_d = nc.dram_tensor("Mh", [D, D], BF16, kind="ExternalInput")
    n0_d = nc.dram_tensor("N0h", [D, D], BF16, kind="ExternalInput")
    bo2_d = nc.dram_tensor("bo2_bc", [128, D], F32, kind="ExternalInput")
    mask_d = nc.dram_tensor("mask", [128, 128], BF16, kind="ExternalInput")
    ones_d = nc.dram_tensor("ones1", [128, 1], BF16, kind="ExternalInput")
    out_d = nc.dram_tensor("out", [SPC, P, D], F32, kind="ExternalOutput")

    with TileContext(nc) as tc:
        with (
            tc.tile_pool(name="consts", bufs=1) as cpool,
            tc.tile_pool(name="xin", bufs=3) as xpool,
            tc.tile_pool(name="proj", bufs=3) as ppool,
            tc.tile_pool(name="attn", bufs=3) as apool,
            tc.tile_pool(name="outp", bufs=2) as opool,
            tc.tile_pool(name="small", bufs=3) as spool,
            tc.tile_pool(name="psum", bufs=7, space="PSUM") as mmpool,
            tc.tile_pool(name="psum_sums", bufs=1, space="PSUM") as sumpool,
        ):
            # Mh gates the first matmul — load it (and the small consts)
            # first; N0h/bo2 are consumed late, so they load after slice 0's
            # activations to keep DMA bandwidth on the critical startup path.
            # HAM pre-warm: the PE idles ~8 us at start waiting for the
            # first DMAs; ~20 dummy matmuls on a zeroed scratch tile run in
            # that window and flip the PE clock-gate to 8/8 (2.4 GHz) before
            # the first real matmul, saving the ~2-3 us cold-rate penalty.
            wu_sb = cpool.tile([128, 512], BF16, tag="warmup")
            nc.vector.memset(wu_sb[:], 0.0)
            ps_wu = mmpool.tile([128, P], F32, tag="mm")
            for w in range(20):
                nc.tensor.matmul(ps_wu[:], lhsT=wu_sb[:, 0:128], rhs=wu_sb[:],
                                 start=w == 0, stop=w == 19)

            m_sb = _load_w(nc, cpool, m_d, split=True)
            mask_sb = cpool.tile([128, 128], BF16, tag="mask")
            nc.sync.dma_start(out=mask_sb[:], in_=mask_d.ap())
            ones_sb = cpool.tile([128, 1], BF16, tag="ones1")
            nc.sync.dma_start(out=ones_sb[:], in_=ones_d.ap())

            state = {}

            def front(s):
                """Loads, u, scoresT/exp/mask, denominators for slice s."""
                xq = _load_x(nc, xpool, xq_d, s, "xq", split=s == 0)
                xk = _load_x(nc, xpool, xk_d, s, "xk")
                xv = _load_x(nc, xpool, xv_d, s, "xv")  # natural [s, d] tiles
                if s == 0:
                    state["n0_sb"] = _load_w(nc, cpool, n0_d)
                    bo2_sb = cpool.tile([128, D], F32, tag="bo2")
                    nc.sync.dma_start(out=bo2_sb[:], in_=bo2_d.ap())
                    state["bo2_sb"] = bo2_sb

                # u = M.T-tiles @ XqT
                u = ppool.tile([128, KT, P], BF16, tag="u")  # [d1, p]
                for m in range(KT):
                    ps = mmpool.tile([128, P], F32, tag="mm")
                    for kk in range(KT):
                        nc.tensor.matmul(
                            ps[:], lhsT=m_sb[:, kk, 128 * m : 128 * (m + 1)],
                            rhs=xq[:, kk, :], start=kk == 0, stop=kk == KT - 1)
                    # split evacuation across two engines to free the PSUM
                    # slot sooner
                    nc.scalar.copy(u[:, m, 0 : P // 2], ps[:, 0 : P // 2])
                    nc.vector.tensor_copy(u[:, m, P // 2 :], ps[:, P // 2 :])

                # scoresT + exp + diag mask
                eT = apool.tile([128, PT, P], BF16, tag="eT")  # [s, p]
                for i in range(PT):
                    ps = mmpool.tile([128, P], F32, tag="mm")
                    for kk in range(KT):
                        nc.tensor.matmul(
                            ps[:], lhsT=xk[:, kk, 128 * i : 128 * (i + 1)],
                            rhs=u[:, kk, :], start=kk == 0, stop=kk == KT - 1)
                    nc.scalar.activation(eT[:, i, :], ps[:], AF.Exp)
                    nc.vector.tensor_mul(
                        eT[:, i, 128 * i : 128 * (i + 1)],
                        eT[:, i, 128 * i : 128 * (i + 1)],
                        mask_sb[:],
                    )

                return (s, xv, eT)

            def back(st):
                """axT and final projection for a previously-fronted slice."""
                s, xv, eT = st
                axT = apool.tile([128, KT, P], BF16, tag="axT")  # [d, p]
                # the denominator (sums) matmul pairs are interleaved with the
                # axT groups: their 128-col LDWEIGHTS ride the LDW queue under
                # the axT matmuls' 213 ns streams instead of serializing the
                # PE in a block of their own
                ps_sum = sumpool.tile([128, PT], F32, tag="sums")
                for m in range(KT):
                    ps = mmpool.tile([128, P], F32, tag="mm")
                    for i in range(PT):
                        nc.tensor.matmul(
                            ps[:], lhsT=xv[:, i, 128 * m : 128 * (m + 1)],
                            rhs=eT[:, i, :], start=i == 0, stop=i == PT - 1)
                    for i in range(PT):
                        nc.tensor.matmul(
                            ps_sum[:, m : m + 1],
                            lhsT=eT[:, i, 128 * m : 128 * (m + 1)],
                            rhs=ones_sb[:], start=i == 0, stop=i == PT - 1)
                    nc.scalar.copy(axT[:, m, 0 : P // 2], ps[:, 0 : P // 2])
                    nc.vector.tensor_copy(axT[:, m, P // 2 :], ps[:, P // 2 :])
                rcpT = spool.tile([128, PT], F32, tag="rcpT")
                nc.vector.reciprocal(rcpT[:], ps_sum[:])

                ot = opool.tile([128, PT, D], F32, tag="ot")
                for j in range(PT):
                    ps = mmpool.tile([128, D], F32, tag="mm")
                    for m in range(KT):
                        nc.tensor.matmul(
                            ps[:], lhsT=axT[:, m, 128 * j : 128 * (j + 1)],
                            rhs=state["n0_sb"][:, m, :],
                            start=m == 0, stop=m == KT - 1)
                    nc.vector.scalar_tensor_tensor(
                        ot[:, j, :], ps[:], rcpT[:, j : j + 1], state["bo2_sb"][:],
                        ALU.mult, ALU.add,
                    )
                    # per-j store so the final DMA overlaps the epilogue
                    nc.sync.dma_start(
                        out=out_d.ap()[s, 128 * j : 128 * (j + 1), :],
                        in_=ot[:, j, :],
                    )

            # Software-pipelined emission: slice s+1's front half is emitted
            # before slice s's back half, so the PE's in-order stream has
            # cross-slice matmuls to run during each slice's exp bubble.
            prev = None
            for s in range(SPC):
                st = front(s)
                if prev is not None:
                    back(prev)
                prev = st
            back(prev)

    nc.compile()
    return nc


def build_program_general():
    """General path with explicit q/k/v projections (nonzero biases)."""
    nc = _new_nc()

    xq_d = nc.dram_tensor("xqT", [SPC, D, P], BF16, kind="ExternalInput")
    xk_d = nc.dram_tensor("xkT", [SPC, D, P], BF16, kind="ExternalInput")
    xv_d = nc.dram_tensor("xvT", [SPC, D, P], BF16, kind="ExternalInput")
    wq_d = nc.dram_tensor("wqT", [D, D], BF16, kind="ExternalInput")
    wkv_d = nc.dram_tensor("wkvT", [D, D], BF16, kind="ExternalInput")
    wo_d = nc.dram_tensor("woT", [D, D], BF16, kind="ExternalInput")
    bq_d = nc.dram_tensor("bq_col", [128, KT], F32, kind="ExternalInput")
    bkv_d = nc.dram_tensor("bkv_col", [128, KT], F32, kind="ExternalInput")
    bkvb_d = nc.dram_tensor("bkv_bc", [128, D], F32, kind="ExternalInput")
    bob_d = nc.dram_tensor("bo_bc", [128, D], F32, kind="ExternalInput")
    mask_d = nc.dram_tensor("mask", [128, 128], BF16, kind="ExternalInput")
    ones_d = nc.dram_tensor("ones1", [128, 1], BF16, kind="ExternalInput")
    out_d = nc.dram_tensor("out", [SPC, P, D], F32, kind="ExternalOutput")

    with TileContext(nc) as tc:
        with (
            tc.tile_pool(name="consts", bufs=1) as cpool,
            tc.tile_pool(name="xin", bufs=2) as xpool,
            tc.tile_pool(name="proj", bufs=2) as ppool,
            tc.tile_pool(name="attn", bufs=2) as apool,
            tc.tile_pool(name="outp", bufs=2) as opool,
            tc.tile_pool(name="small", bufs=2) as spool,
            tc.tile_pool(name="psum", bufs=6, space="PSUM") as mmpool,
            tc.tile_pool(name="psum_sums", bufs=2, space="PSUM") as sumpool,
        ):
            wq_sb = _load_w(nc, cpool, wq_d)
            wkv_sb = _load_w(nc, cpool, wkv_d)
            wo_sb = _load_w(nc, cpool, wo_d)
            bq_sb = cpool.tile([128, KT], F32, tag="bq")
            nc.sync.dma_start(out=bq_sb[:], in_=bq_d.ap())
            bkv_sb = cpool.tile([128, KT], F32, tag="bkv")
            nc.sync.dma_start(out=bkv_sb[:], in_=bkv_d.ap())
            bkvb_sb = cpool.tile([128, D], F32, tag="bkvb")
            nc.sync.dma_start(out=bkvb_sb[:], in_=bkvb_d.ap())
            bob_sb = cpool.tile([128, D], F32, tag="bob")
            nc.sync.dma_start(out=bob_sb[:], in_=bob_d.ap())
            mask_sb = cpool.tile([128, 128], BF16, tag="mask")
            nc.sync.dma_start(out=mask_sb[:], in_=mask_d.ap())
            ones_sb = cpool.tile([128, 1], BF16, tag="ones1")
            nc.sync.dma_start(out=ones_sb[:], in_=ones_d.ap())

            for s in range(SPC):
                xq = _load_x(nc, xpool, xq_d, s, "xq")
                xk = _load_x(nc, xpool, xk_d, s, "xk")
                xv = _load_x(nc, xpool, xv_d, s, "xv")

                qT = ppool.tile([128, KT, P], BF16, tag="qT")  # [dout, p]
                kTt = ppool.tile([128, KT, P], BF16, tag="kT")  # [dout, s]
                vn = ppool.tile([128, PT, D], BF16, tag="vn")  # [s, dout]
                for m in range(KT):
                    ps = mmpool.tile([128, P], F32, tag="mm")
                    for kk in range(KT):
                        nc.tensor.matmul(
                            ps[:], lhsT=wq_sb[:, kk, 128 * m : 128 * (m + 1)],
                            rhs=xq[:, kk, :], start=kk == 0, stop=kk == KT - 1)
                    nc.scalar.activation(qT[:, m, :], ps[:], AF.Identity,
                                         bias=bq_sb[:, m : m + 1])
                for m in range(KT):
                    ps = mmpool.tile([128, P], F32, tag="mm")
                    for kk in range(KT):
                        nc.tensor.matmul(
                            ps[:], lhsT=wkv_sb[:, kk, 128 * m : 128 * (m + 1)],
                            rhs=xk[:, kk, :], start=kk == 0, stop=kk == KT - 1)
                    nc.scalar.activation(kTt[:, m, :], ps[:], AF.Identity,
                                         bias=bkv_sb[:, m : m + 1])
                for i in range(PT):
                    ps = mmpool.tile([128, D], F32, tag="mm")
                    for kk in range(KT):
                        nc.tensor.matmul(
                            ps[:], lhsT=xv[:, kk, 128 * i : 128 * (i + 1)],
                            rhs=wkv_sb[:, kk, :], start=kk == 0, stop=kk == KT - 1)
                    nc.vector.tensor_add(vn[:, i, :], ps[:], bkvb_sb[:])

                eT = apool.tile([128, PT, P], BF16, tag="eT")  # [s, p]
                for i in range(PT):
                    ps = mmpool.tile([128, P], F32, tag="mm")
                    for kk in range(KT):
                        nc.tensor.matmul(
                            ps[:], lhsT=kTt[:, kk, 128 * i : 128 * (i + 1)],
                            rhs=qT[:, kk, :], start=kk == 0, stop=kk == KT - 1)
                    nc.scalar.activation(eT[:, i, :], ps[:], AF.Exp)
                    nc.vector.tensor_mul(
                        eT[:, i, 128 * i : 128 * (i + 1)],
                        eT[:, i, 128 * i : 128 * (i + 1)],
                        mask_sb[:],
                    )

                ps_sum = sumpool.tile([128, PT], F32, tag="sums")
                for j in range(PT):
                    for i in range(PT):
                        nc.tensor.matmul(
                            ps_sum[:, j : j + 1],
                            lhsT=eT[:, i, 128 * j : 128 * (j + 1)],
                            rhs=ones_sb[:], start=i == 0, stop=i == PT - 1)
                rcpT = spool.tile([128, PT], F32, tag="rcpT")
                nc.vector.reciprocal(rcpT[:], ps_sum[:])

                avT = apool.tile([128, KT, P], BF16, tag="avT")  # [dv, p]
                for m in range(KT):
                    ps = mmpool.tile([128, P], F32, tag="mm")
                    for i in range(PT):
                        nc.tensor.matmul(
                            ps[:], lhsT=vn[:, i, 128 * m : 128 * (m + 1)],
                            rhs=eT[:, i, :], start=i == 0, stop=i == PT - 1)
                    nc.scalar.copy(avT[:, m, :], ps[:])

                ot = opool.tile([128, PT, D], F32, tag="ot")
                for j in range(PT):
                    ps = mmpool.tile([128, D], F32, tag="mm")
                    for m in range(KT):
                        nc.tensor.matmul(
                            ps[:], lhsT=avT[:, m, 128 * j : 128 * (j + 1)],
                            rhs=wo_sb[:, m, :], start=m == 0, stop=m == KT - 1)
                    nc.vector.scalar_tensor_tensor(
                        ot[:, j, :], ps[:], rcpT[:, j : j + 1], bob_sb[:],
                        ALU.mult, ALU.add,
                    )
                nc.sync.dma_start(
                    out=out_d.ap()[s].rearrange("(j p) f -> p j f", p=128),
                    in_=ot[:],
                )

    nc.compile()
    return nc


def _bf16(a):
    return np.ascontiguousarray(a).astype(ml_dtypes.bfloat16)


def _norm_inputs(queries, keys, values, Wq, bq, Wkv, bkv, Wo, bo):
    return (
        np.asarray(queries, np.float32).reshape(SLICES, P, D),
        np.asarray(keys, np.float32).reshape(SLICES, P, D),
        np.asarray(values, np.float32).reshape(SLICES, P, D),
        np.asarray(Wq, np.float32), np.asarray(bq, np.float32),
        np.asarray(Wkv, np.float32), np.asarray(bkv, np.float32),
        np.asarray(Wo, np.float32), np.asarray(bo, np.float32),
    )


def prep_in_maps_fast(queries, keys, values, Wq, bq, Wkv, bkv, Wo, bo):
    queries, keys, values, Wq, bq, Wkv, bkv, Wo, bo = _norm_inputs(
        queries, keys, values, Wq, bq, Wkv, bkv, Wo, bo)

    scale = np.float32(1.0 / np.sqrt(D))
    # scores.T = Xk @ M.T @ Xq.T with M[d2,d1] = (scale*Wq).T @ Wkv
    Mh = _bf16((Wq * scale).T @ Wkv)           # [d2, d1]
    N0h = _bf16(Wkv.T @ Wo.T)                  # [d, dout]
    bo2 = Wo @ bkv + bo
    bo2_bc = np.ascontiguousarray(np.broadcast_to(bo2, (128, D))).astype(np.float32)
    mask = _bf16(1.0 - np.eye(128, dtype=np.float32))

    qT = _bf16(queries.transpose(0, 2, 1))     # [slices, D, P]
    kT = _bf16(keys.transpose(0, 2, 1))
    vN = _bf16(values)                         # natural [slices, P, D]

    in_maps = []
    for c in range(N_CORES):
        sl = slice(c * SPC, (c + 1) * SPC)
        in_maps.append({
            "xqT": qT[sl], "xkT": kT[sl], "xvN": vN[sl],
            "Mh": Mh, "N0h": N0h, "bo2_bc": bo2_bc, "mask": mask,
            "ones1": np.ones((128, 1), ml_dtypes.bfloat16),
        })
    return in_maps


def prep_in_maps_general(queries, keys, values, Wq, bq, Wkv, bkv, Wo, bo):
    queries, keys, values, Wq, bq, Wkv, bkv, Wo, bo = _norm_inputs(
        queries, keys, values, Wq, bq, Wkv, bkv, Wo, bo)

    scale = np.float32(1.0 / np.sqrt(D))
    wqT = _bf16((Wq * scale).T)
    wkvT = _bf16(Wkv.T)
    woT = _bf16(Wo.T)
    bq_col = np.ascontiguousarray((bq * scale).reshape(KT, 128).T)
    bkv_col = np.ascontiguousarray(bkv.reshape(KT, 128).T)
    bkv_bc = np.ascontiguousarray(np.broadcast_to(bkv, (128, D))).astype(np.float32)
    bo_bc = np.ascontiguousarray(np.broadcast_to(bo, (128, D))).astype(np.float32)
    mask = _bf16(1.0 - np.eye(128, dtype=np.float32))

    qT = _bf16(queries.transpose(0, 2, 1))
    kT = _bf16(keys.transpose(0, 2, 1))
    vT = _bf16(values.transpose(0, 2, 1))

    in_maps = []
    for c in range(N_CORES):
        sl = slice(c * SPC, (c + 1) * SPC)
        in_maps.append({
            "xqT": qT[sl], "xkT": kT[sl], "xvT": vT[sl],
            "wqT": wqT, "wkvT": wkvT, "woT": woT,
            "bq_col": bq_col, "bkv_col": bkv_col,
            "bkv_bc": bkv_bc, "bo_bc": bo_bc, "mask": mask,
            "ones1": np.ones((128, 1), ml_dtypes.bfloat16),
        })
    return in_maps


_nc_fast = None
_nc_general = None


def kernel(**inputs):
    global _nc_fast, _nc_general
    bq = np.asarray(inputs["bq"], np.float32)
    bkv = np.asarray(inputs["bkv"], np.float32)
    fast = not (np.any(bq) or np.any(bkv))
    if fast:
        if _nc_fast is None:
            _nc_fast = build_program_fast()
        nc, in_maps = _nc_fast, prep_in_maps_fast(**inputs)
    else:
        if _nc_general is None:
            _nc_general = build_program_general()
        nc, in_maps = _nc_general, prep_in_maps_general(**inputs)
    res = run_bass_kernel_spmd(nc, in_maps, core_ids=list(range(N_CORES)))
    out = np.concatenate([res.results[c]["out"] for c in range(N_CORES)], axis=0)
    return out.reshape(B, V, P, D)


# revision 36
# speedup vs baseline: 1.0068x; 1.0068x over previous
"""Trainium2 Bass kernel for per-(b,v)-slice masked attention.

Reference computation (per (b,v) slice, P=S=512, D=512):
    q = X_q @ Wq.T + bq          (softmax scale folded into Wq here)
    k = X_k @ Wkv.T + bkv
    v = X_v @ Wkv.T + bkv
    scores = q @ k.T, diag masked, attn = softmax(scores)
    out = (attn @ v) @ Wo.T + bo

Sharding: 128 (b,v) slices split 16-per-core across 8 cores; weights
replicated. The host pre-transposes q/k activations to d-major layout so
every on-chip matmul contracts over the partition dimension.

Fast path (bq == bkv == 0, which setup_inputs guarantees): fold the
weight products on the host —
    M  = (scale*Wq).T @ Wkv     so  scores.T = Xk @ M.T @ Xq.T
    N0 = Wkv.T @ Wo.T           so  out = (attn @ Xv) @ N0 + (Wo@bkv + bo)
eliminating the k and v projections (4 big matmul groups per slice
instead of 6). Per slice:
    u[d,p]    = (M tiles).T @ XqT
    sT[s,p]   = (XkT tiles).T @ u       (scores transposed)
    eT[s,p]   = exp(sT) * (1 - I)      (diag mask, multiplicative)
    sums[p]   = (eT tiles).T @ ones    (softmax denominator, [128,4] psum)
    axT[d,p]  = (Xv tiles).T @ eT      (attn @ Xv, transposed, unnormalized)
    out[p,o]  = ((axT tiles).T @ N0) * recip(sums)[p] + bo2

General path (nonzero bq/bkv): explicit q/k/v projections as above.

Matmul operands are bf16 (PE streams 1 cycle/row with fast weight load);
all PSUM accumulation and softmax normalization stay fp32.
"""

import numpy as np
import ml_dtypes

import concourse.bacc as bacc
import concourse.mybir as mybir
from concourse.tile import TileContext
from concourse.bass_utils import run_bass_kernel_spmd

B, V, P, D = 4, 32, 512, 512
N_CORES = 8
SLICES = B * V  # 128
SPC = SLICES // N_CORES  # 16 slices per core
KT = D // 128  # 4 contraction tiles
PT = P // 128  # 4 token tiles

BF16 = mybir.dt.bfloat16
F32 = mybir.dt.float32
AF = mybir.ActivationFunctionType
ALU = mybir.AluOpType


def _new_nc():
    return bacc.Bacc("TRN2", target_bir_lowering=False, debug=False,
                     num_devices=N_CORES)


def _load_w(nc, cpool, dram, split=False):
    t = cpool.tile([128, KT, D], BF16, tag=dram.name)
    src = dram.ap().rearrange("(kk p) f -> p kk f", p=128)
    if split:
        # per-kk pieces release matmul dependencies as each chunk lands
        for kk in range(KT):
            nc.sync.dma_start(out=t[:, kk, :], in_=src[:, kk, :])
    else:
        nc.sync.dma_start(out=t[:], in_=src)
    return t


def _load_x(nc, xpool, dram, s, tag, split=False, engine=None):
    t = xpool.tile([128, KT, P], BF16, tag=tag)
    src = dram.ap()[s].rearrange("(kk p) f -> p kk f", p=128)
    if split:
        for kk in range(KT):
            nc.sync.dma_start(out=t[:, kk, :], in_=src[:, kk, :])
    else:
        (engine or nc.sync).dma_start(out=t[:], in_=src)
    return t


def build_program_fast():
    """Zero-bias fast path: 4 matmul groups per slice."""
    nc = _new_nc()

    xq_d = nc.dram_tensor("xqT", [SPC, D, P], BF16, kind="ExternalInput")
    xk_d = nc.dram_tensor("xkT", [SPC, D, P], BF16, kind="ExternalInput")
    xv_d = nc.dram_tensor("xvN", [SPC, P, D], BF16, kind="ExternalInput")
    m_d = nc.dram_tensor("Mh", [D, D], BF16, kind="ExternalInput")
    n0_d = nc.dram_tensor("N0h", [D, D], BF16, kind="ExternalInput")
    bo2_d = nc.dram_tensor("bo2_bc", [128, D], F32, kind="ExternalInput")
    mask_d = nc.dram_tensor("mask", [128, 128], BF16, kind="ExternalInput")
    ones_d = nc.dram_tensor("ones1", [128, 1], BF16, kind="ExternalInput")
    out_d = nc.dram_tensor("out", [SPC, P, D], F32, kind="ExternalOutput")

    with TileContext(nc) as tc:
        with (
            tc.tile_pool(name="consts", bufs=1) as cpool,
            tc.tile_pool(name="xin", bufs=3) as xpool,
            tc.tile_pool(name="proj", bufs=3) as ppool,
            tc.tile_pool(name="attn", bufs=3) as apool,
            tc.tile_pool(name="outp", bufs=2) as opool,
            tc.tile_pool(name="small", bufs=3) as spool,
            tc.tile_pool(name="psum", bufs=7, space="PSUM") as mmpool,
            tc.tile_pool(name="psum_sums", bufs=1, space="PSUM") as sumpool,
        ):
            # Mh gates the first matmul — load it (and the small consts)
            # first; N0h/bo2 are consumed late, so they load after slice 0's
            # activations to keep DMA bandwidth on the critical startup path.
            m_sb = _load_w(nc, cpool, m_d, split=True)
            mask_sb = cpool.tile([128, 128], BF16, tag="mask")
            nc.sync.dma_start(out=mask_sb[:], in_=mask_d.ap())
            ones_sb = cpool.tile([128, 1], BF16, tag="ones1")
            nc.sync.dma_start(out=ones_sb[:], in_=ones_d.ap())

            state = {}

            def front(s):
                """Loads, u, scoresT/exp/mask, denominators for slice s."""
                xq = _load_x(nc, xpool, xq_d, s, "xq", split=s == 0)
                xk = _load_x(nc, xpool, xk_d, s, "xk")
                xv = _load_x(nc, xpool, xv_d, s, "xv")  # natural [s, d] tiles
                if s == 0:
                    state["n0_sb"] = _load_w(nc, cpool, n0_d)
                    bo2_sb = cpool.tile([128, D], F32, tag="bo2")
                    nc.sync.dma_start(out=bo2_sb[:], in_=bo2_d.ap())
                    state["bo2_sb"] = bo2_sb

                # u = M.T-tiles @ XqT
                u = ppool.tile([128, KT, P], BF16, tag="u")  # [d1, p]
                for m in range(KT):
                    ps = mmpool.tile([128, P], F32, tag="mm")
                    for kk in range(KT):
                        nc.tensor.matmul(
                            ps[:], lhsT=m_sb[:, kk, 128 * m : 128 * (m + 1)],
                            rhs=xq[:, kk, :], start=kk == 0, stop=kk == KT - 1)
                    # split evacuation across two engines to free the PSUM
                    # slot sooner
                    nc.scalar.copy(u[:, m, 0 : P // 2], ps[:, 0 : P // 2])
                    nc.vector.tensor_copy(u[:, m, P // 2 :], ps[:, P // 2 :])

                # scoresT + exp + diag mask
                eT = apool.tile([128, PT, P], BF16, tag="eT")  # [s, p]
                for i in range(PT):
                    ps = mmpool.tile([128, P], F32, tag="mm")
                    for kk in range(KT):
                        nc.tensor.matmul(
                            ps[:], lhsT=xk[:, kk, 128 * i : 128 * (i + 1)],
                            rhs=u[:, kk, :], start=kk == 0, stop=kk == KT - 1)
                    nc.scalar.activation(eT[:, i, :], ps[:], AF.Exp)
                    nc.vector.tensor_mul(
                        eT[:, i, 128 * i : 128 * (i + 1)],
                        eT[:, i, 128 * i : 128 * (i + 1)],
                        mask_sb[:],
                    )

                return (s, xv, eT)

            def back(st):
                """axT and final projection for a previously-fronted slice."""
                s, xv, eT = st
                axT = apool.tile([128, KT, P], BF16, tag="axT")  # [d, p]
                # the denominator (sums) matmul pairs are interleaved with the
                # axT groups: their 128-col LDWEIGHTS ride the LDW queue under
                # the axT matmuls' 213 ns streams instead of serializing the
                # PE in a block of their own
                ps_sum = sumpool.tile([128, PT], F32, tag="sums")
                for m in range(KT):
                    ps = mmpool.tile([128, P], F32, tag="mm")
                    for i in range(PT):
                        nc.tensor.matmul(
                            ps[:], lhsT=xv[:, i, 128 * m : 128 * (m + 1)],
                            rhs=eT[:, i, :], start=i == 0, stop=i == PT - 1)
                    for i in range(PT):
                        nc.tensor.matmul(
                            ps_sum[:, m : m + 1],
                            lhsT=eT[:, i, 128 * m : 128 * (m + 1)],
                            rhs=ones_sb[:], start=i == 0, stop=i == PT - 1)
                    nc.scalar.copy(axT[:, m, 0 : P // 2], ps[:, 0 : P // 2])
                    nc.vector.tensor_copy(axT[:, m, P // 2 :], ps[:, P // 2 :])
                rcpT = spool.tile([128, PT], F32, tag="rcpT")
                nc.vector.reciprocal(rcpT[:], ps_sum[:])

                ot = opool.tile([128, PT, D], F32, tag="ot")
                for j in range(PT):
                    ps = mmpool.tile([128, D], F32, tag="mm")
                    for m in range(KT):
                        nc.tensor.matmul(
                            ps[:], lhsT=axT[:, m, 128 * j : 128 * (j + 1)],
                            rhs=state["n0_sb"][:, m, :],
                            start=m == 0, stop=m == KT - 1)
                    nc.vector.scalar_tensor_tensor(
                        ot[:, j, :], ps[:], rcpT[:, j : j + 1], state["bo2_sb"][:],
                        ALU.mult, ALU.add,
                    )
                    # per-j store so the final DMA overlaps the epilogue
                    nc.sync.dma_start(
                        out=out_d.ap()[s, 128 * j : 128 * (j + 1), :],
                        in_=ot[:, j, :],
                    )

            # Software-pipelined emission: slice s+1's front half is emitted
            # before slice s's back half, so the PE's in-order stream has
            # cross-slice matmuls to run during each slice's exp bubble.
            prev = None
            for s in range(SPC):
                st = front(s)
                if prev is not None:
                    back(prev)
                prev = st
            back(prev)

    nc.compile()
    return nc


def build_program_general():
    """General path with explicit q/k/v projections (nonzero biases)."""
    nc = _new_nc()

    xq_d = nc.dram_tensor("xqT", [SPC, D, P], BF16, kind="ExternalInput")
    xk_d = nc.dram_tensor("xkT", [SPC, D, P], BF16, kind="ExternalInput")
    xv_d = nc.dram_tensor("xvT", [SPC, D, P], BF16, kind="ExternalInput")
    wq_d = nc.dram_tensor("wqT", [D, D], BF16, kind="ExternalInput")
    wkv_d = nc.dram_tensor("wkvT", [D, D], BF16, kind="ExternalInput")
    wo_d = nc.dram_tensor("woT", [D, D], BF16, kind="ExternalInput")
    bq_d = nc.dram_tensor("bq_col", [128, KT], F32, kind="ExternalInput")
    bkv_d = nc.dram_tensor("bkv_col", [128, KT], F32, kind="ExternalInput")
    bkvb_d = nc.dram_tensor("bkv_bc", [128, D], F32, kind="ExternalInput")
    bob_d = nc.dram_tensor("bo_bc", [128, D], F32, kind="ExternalInput")
    mask_d = nc.dram_tensor("mask", [128, 128], BF16, kind="ExternalInput")
    ones_d = nc.dram_tensor("ones1", [128, 1], BF16, kind="ExternalInput")
    out_d = nc.dram_tensor("out", [SPC, P, D], F32, kind="ExternalOutput")

    with TileContext(nc) as tc:
        with (
            tc.tile_pool(name="consts", bufs=1) as cpool,
            tc.tile_pool(name="xin", bufs=2) as xpool,
            tc.tile_pool(name="proj", bufs=2) as ppool,
            tc.tile_pool(name="attn", bufs=2) as apool,
            tc.tile_pool(name="outp", bufs=2) as opool,
            tc.tile_pool(name="small", bufs=2) as spool,
            tc.tile_pool(name="psum", bufs=6, space="PSUM") as mmpool,
            tc.tile_pool(name="psum_sums", bufs=2, space="PSUM") as sumpool,
        ):
            wq_sb = _load_w(nc, cpool, wq_d)
            wkv_sb = _load_w(nc, cpool, wkv_d)
            wo_sb = _load_w(nc, cpool, wo_d)
            bq_sb = cpool.tile([128, KT], F32, tag="bq")
            nc.sync.dma_start(out=bq_sb[:], in_=bq_d.ap())
            bkv_sb = cpool.tile([128, KT], F32, tag="bkv")
            nc.sync.dma_start(out=bkv_sb[:], in_=bkv_d.ap())
            bkvb_sb = cpool.tile([128, D], F32, tag="bkvb")
            nc.sync.dma_start(out=bkvb_sb[:], in_=bkvb_d.ap())
            bob_sb = cpool.tile([128, D], F32, tag="bob")
            nc.sync.dma_start(out=bob_sb[:], in_=bob_d.ap())
            mask_sb = cpool.tile([128, 128], BF16, tag="mask")
            nc.sync.dma_start(out=mask_sb[:], in_=mask_d.ap())
            ones_sb = cpool.tile([128, 1], BF16, tag="ones1")
            nc.sync.dma_start(out=ones_sb[:], in_=ones_d.ap())

            for s in range(SPC):
                xq = _load_x(nc, xpool, xq_d, s, "xq")
                xk = _load_x(nc, xpool, xk_d, s, "xk")
                xv = _load_x(nc, xpool, xv_d, s, "xv")

                qT = ppool.tile([128, KT, P], BF16, tag="qT")  # [dout, p]
                kTt = ppool.tile([128, KT, P], BF16, tag="kT")  # [dout, s]
                vn = ppool.tile([128, PT, D], BF16, tag="vn")  # [s, dout]
                for m in range(KT):
                    ps = mmpool.tile([128, P], F32, tag="mm")
                    for kk in range(KT):
                        nc.tensor.matmul(
                            ps[:], lhsT=wq_sb[:, kk, 128 * m : 128 * (m + 1)],
                            rhs=xq[:, kk, :], start=kk == 0, stop=kk == KT - 1)
                    nc.scalar.activation(qT[:, m, :], ps[:], AF.Identity,
                                         bias=bq_sb[:, m : m + 1])
                for m in range(KT):
                    ps = mmpool.tile([128, P], F32, tag="mm")
                    for kk in range(KT):
                        nc.tensor.matmul(
                            ps[:], lhsT=wkv_sb[:, kk, 128 * m : 128 * (m + 1)],
                            rhs=xk[:, kk, :], start=kk == 0, stop=kk == KT - 1)
                    nc.scalar.activation(kTt[:, m, :], ps[:], AF.Identity,
                                         bias=bkv_sb[:, m : m + 1])
                for i in range(PT):
                    ps = mmpool.tile([128, D], F32, tag="mm")
                    for kk in range(KT):
                        nc.tensor.matmul(
                            ps[:], lhsT=xv[:, kk, 128 * i : 128 * (i + 1)],
                            rhs=wkv_sb[:, kk, :], start=kk == 0, stop=kk == KT - 1)
                    nc.vector.tensor_add(vn[:, i, :], ps[:], bkvb_sb[:])

                eT = apool.tile([128, PT, P], BF16, tag="eT")  # [s, p]
                for i in range(PT):
                    ps = mmpool.tile([128, P], F32, tag="mm")
                    for kk in range(KT):
                        nc.tensor.matmul(
                            ps[:], lhsT=kTt[:, kk, 128 * i : 128 * (i + 1)],
                            rhs=qT[:, kk, :], start=kk == 0, stop=kk == KT - 1)
                    nc.scalar.activation(eT[:, i, :], ps[:], AF.Exp)
                    nc.vector.tensor_mul(
                        eT[:, i, 128 * i : 128 * (i + 1)],
                        eT[:, i, 128 * i : 128 * (i + 1)],
                        mask_sb[:],
                    )

                ps_sum = sumpool.tile([128, PT], F32, tag="sums")
                for j in range(PT):
                    for i in range(PT):
                        nc.tensor.matmul(
                            ps_sum[:, j : j + 1],
                            lhsT=eT[:, i, 128 * j : 128 * (j + 1)],
                            rhs=ones_sb[:], start=i == 0, stop=i == PT - 1)
                rcpT = spool.tile([128, PT], F32, tag="rcpT")
                nc.vector.reciprocal(rcpT[:], ps_sum[:])

                avT = apool.tile([128, KT, P], BF16, tag="avT")  # [dv, p]
                for m in range(KT):
                    ps = mmpool.tile([128, P], F32, tag="mm")
                    for i in range(PT):
                        nc.tensor.matmul(
                            ps[:], lhsT=vn[:, i, 128 * m : 128 * (m + 1)],
                            rhs=eT[:, i, :], start=i == 0, stop=i == PT - 1)
                    nc.scalar.copy(avT[:, m, :], ps[:])

                ot = opool.tile([128, PT, D], F32, tag="ot")
                for j in range(PT):
                    ps = mmpool.tile([128, D], F32, tag="mm")
                    for m in range(KT):
                        nc.tensor.matmul(
                            ps[:], lhsT=avT[:, m, 128 * j : 128 * (j + 1)],
                            rhs=wo_sb[:, m, :], start=m == 0, stop=m == KT - 1)
                    nc.vector.scalar_tensor_tensor(
                        ot[:, j, :], ps[:], rcpT[:, j : j + 1], bob_sb[:],
                        ALU.mult, ALU.add,
                    )
                nc.sync.dma_start(
                    out=out_d.ap()[s].rearrange("(j p) f -> p j f", p=128),
                    in_=ot[:],
                )

    nc.compile()
    return nc


def _bf16(a):
    return np.ascontiguousarray(a).astype(ml_dtypes.bfloat16)


def _norm_inputs(queries, keys, values, Wq, bq, Wkv, bkv, Wo, bo):
    return (
        np.asarray(queries, np.float32).reshape(SLICES, P, D),
        np.asarray(keys, np.float32).reshape(SLICES, P, D),
        np.asarray(values, np.float32).reshape(SLICES, P, D),
        np.asarray(Wq, np.float32), np.asarray(bq, np.float32),
        np.asarray(Wkv, np.float32), np.asarray(bkv, np.float32),
        np.asarray(Wo, np.float32), np.asarray(bo, np.float32),
    )


def prep_in_maps_fast(queries, keys, values, Wq, bq, Wkv, bkv, Wo, bo):
    queries, keys, values, Wq, bq, Wkv, bkv, Wo, bo = _norm_inputs(
        queries, keys, values, Wq, bq, Wkv, bkv, Wo, bo)

    scale = np.float32(1.0 / np.sqrt(D))
    # scores.T = Xk @ M.T @ Xq.T with M[d2,d1] = (scale*Wq).T @ Wkv
    Mh = _bf16((Wq * scale).T @ Wkv)           # [d2, d1]
    N0h = _bf16(Wkv.T @ Wo.T)                  # [d, dout]
    bo2 = Wo @ bkv + bo
    bo2_bc = np.ascontiguousarray(np.broadcast_to(bo2, (128, D))).astype(np.float32)
    mask = _bf16(1.0 - np.eye(128, dtype=np.float32))

    qT = _bf16(queries.transpose(0, 2, 1))     # [slices, D, P]
    kT = _bf16(keys.transpose(0, 2, 1))
    vN = _bf16(values)                         # natural [slices, P, D]

    in_maps = []
    for c in range(N_CORES):
        sl = slice(c * SPC, (c + 1) * SPC)
        in_maps.append({
            "xqT": qT[sl], "xkT": kT[sl], "xvN": vN[sl],
            "Mh": Mh, "N0h": N0h, "bo2_bc": bo2_bc, "mask": mask,
            "ones1": np.ones((128, 1), ml_dtypes.bfloat16),
        })
    return in_maps


def prep_in_maps_general(queries, keys, values, Wq, bq, Wkv, bkv, Wo, bo):
    queries, keys, values, Wq, bq, Wkv, bkv, Wo, bo = _norm_inputs(
        queries, keys, values, Wq, bq, Wkv, bkv, Wo, bo)

    scale = np.float32(1.0 / np.sqrt(D))
    wqT = _bf16((Wq * scale).T)
    wkvT = _bf16(Wkv.T)
    woT = _bf16(Wo.T)
    bq_col = np.ascontiguousarray((bq * scale).reshape(KT, 128).T)
    bkv_col = np.ascontiguousarray(bkv.reshape(KT, 128).T)
    bkv_bc = np.ascontiguousarray(np.broadcast_to(bkv, (128, D))).astype(np.float32)
    bo_bc = np.ascontiguousarray(np.broadcast_to(bo, (128, D))).astype(np.float32)
    mask = _bf16(1.0 - np.eye(128, dtype=np.float32))

    qT = _bf16(queries.transpose(0, 2, 1))
    kT = _bf16(keys.transpose(0, 2, 1))
    vT = _bf16(values.transpose(0, 2, 1))

    in_maps = []
    for c in range(N_CORES):
        sl = slice(c * SPC, (c + 1) * SPC)
        in_maps.append({
            "xqT": qT[sl], "xkT": kT[sl], "xvT": vT[sl],
            "wqT": wqT, "wkvT": wkvT, "woT": woT,
            "bq_col": bq_col, "bkv_col": bkv_col,
            "bkv_bc": bkv_bc, "bo_bc": bo_bc, "mask": mask,
            "ones1": np.ones((128, 1), ml_dtypes.bfloat16),
        })
    return in_maps


_nc_fast = None
_nc_general = None


def kernel(**inputs):
    global _nc_fast, _nc_general
    bq = np.asarray(inputs["bq"], np.float32)
    bkv = np.asarray(inputs["bkv"], np.float32)
    fast = not (np.any(bq) or np.any(bkv))
    if fast:
        if _nc_fast is None:
            _nc_fast = build_program_fast()
        nc, in_maps = _nc_fast, prep_in_maps_fast(**inputs)
    else:
        if _nc_general is None:
            _nc_general = build_program_general()
        nc, in_maps = _nc_general, prep_in_maps_general(**inputs)
    res = run_bass_kernel_spmd(nc, in_maps, core_ids=list(range(N_CORES)))
    out = np.concatenate([res.results[c]["out"] for c in range(N_CORES)], axis=0)
    return out.reshape(B, V, P, D)
